# revision 20
# baseline (speedup 1.0000x reference)
"""Trainium2 Bass kernel: batched dense GAT (PyG GATConv, eval, concat heads).

Contract: kernel(**inputs) takes FULL inputs (numpy/jax arrays) and returns the
FULL output [B, N, H*C] float32. Internally shards across 8 NeuronCores:
core c handles graph b = c//2, target-node half j0 = (c%2)*1024.

Math (per graph):
  x = f @ W                       [N, H, C]
  a_src[i,h] = <x[i,h,:], att_src[h,:]>,  a_dst likewise
  logit[i,j,h] = leaky_relu(a_src[i,h] + a_dst[j,h], 0.2), -inf off-mask
  alpha = softmax over i (sources);  out[j] = sum_i alpha * x[i]  (+bias, ELU)

Algorithm: the softmax over i cancels any per-target factor, so
P'' = P / exp(b_j) is used, with
  exp(lrelu(a+b) - b) = max(e^a, e^{0.2a} e^{-0.8b})
                      = e^{0.2a} * max(e^{0.8a}, e^{-0.8b}).
The per-SOURCE factor e^{0.2a} is folded into the x-matrix
(xs = (x + bias) * e^{0.2a}; the ones-column becomes e^{0.2a} so the
denominator falls out of the same matmul, and folding the output bias into
x works because out = num/den + bias = (num + bias*den)/den).

Per (h, ib) attention tile, two route choices feeding a shared u-buffer:
  TS route (DVE): u = (e1rb[h] max th[i])             tensor_scalar, 2x mode
  R route (ACT):  r = relu(-0.8b - 0.8a) ; u = exp(r + 0.8a)   two ACTIVATEs
then ONE mask multiply per head-PAIR (amortizes DVE op overhead):
  pe2 = u2 * mask                tensor_tensor [P, 2*NJ], 2x mode
The aggregation runs one wide [65, 1024] matmul per (h, ib) accumulating
[num | den] over i-blocks; PE transposes (4 per bank) land in a single PSUM
bank so one strided ACT copy moves 4 j-quads at once; normalization is a
stride-0-broadcast reciprocal multiply at stage 3.

Host precomputes a_src/a_dst (= f @ (W att)) and ships exp families (tiny),
bf16 fT/W, and the bf16 0/1 self-loop mask (no device casts, half the DMA).
"""

import sys

for _p in ("/opt/trn_rl_repo",):
    if _p not in sys.path:
        sys.path.insert(0, _p)

import numpy as np

# Problem dims (fixed by the task)
B, N, D = 4, 2048, 512
H, C = 8, 64
HC = H * C
NCORES = 8
NJ = 1024        # target columns per core
P = 128
NIB = N // P     # 16 source blocks
NJB = NJ // P    # 8 target blocks
KD = D // P      # 4 contraction blocks
SLOPE = 0.2
FB = 66          # per-head xs stride: 64 x-cols + e02 col + pad (4B align)

# (h, ib) tiles routed through the ACT-heavy R route (Relu+Exp with the
# 0.8a terms as ACT biases), spread evenly over heads 2..7 so the all-DVE
# warm-up pair (heads 0-1) can be emitted before stage 1.
N_R_TILES = 34
_r_pool = [(h, ib) for h in range(2, H) for ib in range(1, NIB - 1)]
R_TILES = frozenset(
    _r_pool[(k * len(_r_pool)) // N_R_TILES] for k in range(N_R_TILES)
)

_PROG = None  # cached (nc, input_names)


def _build_program():
    import concourse.bass as bass
    import concourse.mybir as mybir
    import concourse.tile as tile

    f32 = mybir.dt.float32
    bf16 = mybir.dt.bfloat16
    AF = mybir.ActivationFunctionType
    OP = mybir.AluOpType

    nc = bass.Bass("TRN2", target_bir_lowering=False, debug=False)

    fT_d = nc.dram_tensor("fT", [D, N], bf16, kind="ExternalInput").ap()
    W_d = nc.dram_tensor("W", [D, HC], bf16, kind="ExternalInput").ap()
    Wb_d = nc.dram_tensor("Wb", [1, HC], bf16, kind="ExternalInput").ap()
    # 0/1 mask with self-loops ORed in, bf16
    mask_d = nc.dram_tensor("mask", [N, NJ], bf16, kind="ExternalInput").ap()
    # e1r[h, j] = exp(-0.8 a_dst[j,h]);  b08n[h, j] = -0.8 a_dst[j,h]
    e1r_d = nc.dram_tensor("e1r", [H, NJ], bf16, kind="ExternalInput").ap()
    b08n_d = nc.dram_tensor("b08n", [H, NJ], bf16, kind="ExternalInput").ap()
    # per-source scalars, [p, ib*H + h] layout:
    # th = exp(0.8 a_src), e02 = exp(0.2 a_src), a08p/a08n = +/-0.8 a_src
    th_d = nc.dram_tensor("th", [P, NIB * H], f32, kind="ExternalInput").ap()
    e02_d = nc.dram_tensor("e02", [P, NIB * H], f32, kind="ExternalInput").ap()
    a08p_d = nc.dram_tensor("a08p", [P, NIB * H], f32, kind="ExternalInput").ap()
    a08n_d = nc.dram_tensor("a08n", [P, NIB * H], f32, kind="ExternalInput").ap()
    ident_d = nc.dram_tensor("ident", [P, P], f32, kind="ExternalInput").ap()
    out_d = nc.dram_tensor("out", [NJ, HC], f32, kind="ExternalOutput").ap()

    with tile.TileContext(nc) as tc:
        with (
            tc.tile_pool(name="persist", bufs=1) as pp,
            tc.tile_pool(name="psum_nt", bufs=2, space="PSUM") as ntp,
            tc.tile_pool(name="psum_tr", bufs=2, space="PSUM") as trp,
        ):
            # ---------------- persistent inputs ----------------
            # sync (SP) DMA queue: projection weights first (PE critical
            # path), then the big mask tiles.  gpsimd (Pool) queue: all the
            # small/broadcast tensors the DVE/ACT need early.
            th_sb = pp.tile([P, NIB, H], f32, name="th_sb")
            nc.sync.dma_start(
                th_sb[:], th_d.rearrange("p (a h) -> p a h", a=NIB)
            )
            e1rb = [pp.tile([P, NJ], bf16, name=f"e1rb{h}") for h in range(H)]
            for h in range(2):
                nc.sync.dma_start(
                    e1rb[h][:], e1r_d[h:h + 1, :].to_broadcast((P, NJ))
                )
            e02_sb = pp.tile([P, NIB, H], f32, name="e02_sb")
            nc.gpsimd.dma_start(
                e02_sb[:], e02_d.rearrange("p (a h) -> p a h", a=NIB)
            )
            ident = pp.tile([P, P], f32, name="ident")
            nc.gpsimd.dma_start(ident[:], ident_d[:])
            a08p_sb = pp.tile([P, NIB, H], f32, name="a08p_sb")
            nc.gpsimd.dma_start(
                a08p_sb[:], a08p_d.rearrange("p (a h) -> p a h", a=NIB)
            )
            a08n_sb = pp.tile([P, NIB, H], f32, name="a08n_sb")
            nc.gpsimd.dma_start(
                a08n_sb[:], a08n_d.rearrange("p (a h) -> p a h", a=NIB)
            )
            for h in range(2, H):
                nc.gpsimd.dma_start(
                    e1rb[h][:], e1r_d[h:h + 1, :].to_broadcast((P, NJ))
                )
            bneg = [pp.tile([P, NJ], bf16, name=f"bneg{h}") for h in range(2, H)]
            for h in range(2, H):
                nc.gpsimd.dma_start(
                    bneg[h - 2][:], b08n_d[h:h + 1, :].to_broadcast((P, NJ))
                )
            Wb = pp.tile([1, HC], bf16, name="Wb")
            nc.gpsimd.dma_start(Wb[:], Wb_d[:])

            mask_all = pp.tile([P, NIB, NJ], bf16, name="mask_all")

            junk_ps = ntp.tile([1, 1], f32, name="junk_ps", tag="nt")

            # persistent cross-stage tensors
            xs2 = [pp.tile([P, 2, H, FB], bf16, name=f"xs{i}")
                   for i in range(NIB // 2)]
            zt_all = pp.tile([P, NJB, HC], bf16, name="zt_all")
            rec_all = pp.tile([P, H, NJB], bf16, name="rec_all")
            onesr = pp.tile([1, P], bf16, name="onesr")
            nc.vector.memset(onesr[:], 1.0)

            with (
                tc.tile_pool(name="stage1", bufs=1) as s1p,
                tc.tile_pool(name="work1", bufs=3) as wp1,
                tc.tile_pool(name="work2", bufs=3) as wp,
                tc.tile_pool(name="psum_x", bufs=2, space="PSUM") as xp,
            ):
                # sync queue order tuned for pipeline fill: the first two
                # mask tiles (DVE warm-up), then fT COLUMN chunks q=0 for all
                # kd (+W) so projection of the first i-blocks can start
                # before the whole fT lands, then masks interleaved with the
                # remaining fT column chunks.
                fTt = [s1p.tile([P, N], bf16, name=f"ft_{kd}")
                       for kd in range(KD)]
                Wt = [s1p.tile([P, HC], bf16, name=f"w_{kd}")
                      for kd in range(KD)]
                for ib in range(2):
                    nc.sync.dma_start(
                        mask_all[:, ib, :], mask_d[ib * P:(ib + 1) * P, :]
                    )
                NQ = N // 4
                for kd in range(KD):
                    q0 = nc.sync if kd < 2 else nc.scalar
                    q0.dma_start(
                        fTt[kd][:, 0:NQ], fT_d[kd * P:(kd + 1) * P, 0:NQ]
                    )
                    q0.dma_start(Wt[kd][:], W_d[kd * P:(kd + 1) * P, :])
                for q in range(1, 4):
                    for kd in range(KD):
                        nc.scalar.dma_start(
                            fTt[kd][:, q * NQ:(q + 1) * NQ],
                            fT_d[kd * P:(kd + 1) * P, q * NQ:(q + 1) * NQ],
                        )
                for ib in range(2, NIB):
                    nc.sync.dma_start(
                        mask_all[:, ib, :], mask_d[ib * P:(ib + 1) * P, :]
                    )

                # PE-touch the DMA-loaded ident early (single-wait PE rule:
                # later transposes reusing it then need no DMA wait).
                nc.tensor.matmul(
                    junk_ps[:], ident[0:1, 0:1], ident[0:1, 0:1],
                    start=True, stop=True,
                )

                # -------- stage 2 warm-up: heads 0-1 u/pe quads for the
                # first PREQ i-block PAIRS (all-DVE TS route) emitted FIRST
                # so the DVE queue has runway while PE does the projection.
                # PREQ must stay below the pe pool depth: later pe tiles
                # recycle buffers whose consumers (agg matmuls) are emitted
                # after stage 1, so emitting more would deadlock the DVE
                # queue.
                PREQ = 4
                pe_hp0 = []
                for ibp in range(PREQ):
                    u4 = wp.tile([P, 2, 2, NJ], bf16, name="u4", tag="u2",
                                 bufs=3)
                    for ki in range(2):
                        for k in range(2):
                            nc.vector.tensor_scalar(
                                out=u4[:, ki, k, :],
                                in0=e1rb[k][:],
                                scalar1=th_sb[:, 2 * ibp + ki, k:k + 1],
                                scalar2=None,
                                op0=OP.max,
                                op1=OP.bypass,
                            )
                    pe4 = wp.tile([P, 2, 2, NJ], bf16, name="pe4", tag="pe",
                                  bufs=4)
                    nc.vector.tensor_mul(
                        pe4[:], u4[:],
                        mask_all[:, 2 * ibp:2 * ibp + 2, :].unsqueeze(2)
                        .to_broadcast((P, 2, 2, NJ)),
                    )
                    pe_hp0.append(pe4)

                # ---------------- fused stage 1 + head-pair 0 ----------
                # per i-block: projection matmuls -> xs, then immediately the
                # hp0 aggregation matmuls, so PE never serializes a long
                # projection-only phase while DVE runs ahead.
                nts0 = {
                    h: ntp.tile([65, NJ], f32, name=f"nt{h}", tag="nt")
                    for h in (0, 1)
                }
                for ibp in range(NIB // 2):
                    tmp = wp1.tile([P, 2, H, FB], bf16, name="tmp", tag="tmp")
                    for ki in range(2):
                        ib = 2 * ibp + ki
                        px = xp.tile([P, HC], f32, name="px", tag="px")
                        for kd in range(KD):
                            nc.tensor.matmul(
                                px[:],
                                fTt[kd][:, ib * P:(ib + 1) * P],
                                Wt[kd][:],
                                start=(kd == 0),
                                stop=False,
                            )
                        # bias via rank-1 ones matmul (output bias -> x)
                        nc.tensor.matmul(
                            px[:], onesr[:], Wb[:], start=False, stop=True,
                        )
                        nc.scalar.copy(
                            tmp[:, ki, :, 0:64],
                            px.rearrange("p (h c) -> p h c", h=H),
                        )
                    nc.vector.memset(tmp[:, :, :, 64:FB], 1.0)
                    e02b = e02_sb[:, 2 * ibp:2 * ibp + 2, :].unsqueeze(
                        3).to_broadcast((P, 2, H, FB))
                    xsp = xs2[ibp]
                    nc.vector.tensor_mul(xsp[:], tmp[:], e02b)
                    if ibp < PREQ:
                        pe4 = pe_hp0[ibp]
                    else:
                        u4 = wp.tile([P, 2, 2, NJ], bf16, name="u4",
                                     tag="u2", bufs=3)
                        for ki in range(2):
                            for k in range(2):
                                nc.vector.tensor_scalar(
                                    out=u4[:, ki, k, :],
                                    in0=e1rb[k][:],
                                    scalar1=th_sb[:, 2 * ibp + ki, k:k + 1],
                                    scalar2=None,
                                    op0=OP.max,
                                    op1=OP.bypass,
                                )
                        pe4 = wp.tile([P, 2, 2, NJ], bf16, name="pe4",
                                      tag="pe", bufs=4)
                        nc.vector.tensor_mul(
                            pe4[:], u4[:],
                            mask_all[:, 2 * ibp:2 * ibp + 2, :].unsqueeze(2)
                            .to_broadcast((P, 2, 2, NJ)),
                        )
                    for ki in range(2):
                        for k, h in enumerate((0, 1)):
                            for jc in range(2):
                                nc.tensor.matmul(
                                    nts0[h][:, jc * 512:(jc + 1) * 512],
                                    xsp[:, ki, h, 0:65],
                                    pe4[:, ki, k, jc * 512:(jc + 1) * 512],
                                    start=(ibp == 0 and ki == 0),
                                    stop=(ibp == NIB // 2 - 1 and ki == 1),
                                )

                def _emit_stage3(jts):
                    for jt in jts:
                        recb = rec_all[:, :, jt].unsqueeze(2).to_broadcast(
                            (P, H, C)
                        )
                        zn = wp.tile([P, H, C], bf16, name="zn", tag="zn",
                                     bufs=2)
                        nc.vector.tensor_mul(
                            zn[:], zt_all[:, jt, :].rearrange(
                                "p (h c) -> p h c", h=H), recb,
                        )
                        ee = wp.tile([P, HC], f32, name="ee", tag="ee",
                                     bufs=2)
                        nc.scalar.activation(
                            ee[:], zn.rearrange("p h c -> p (h c)"), AF.Exp
                        )
                        # elu(z) = relu(z) + min(exp(z) - 1, 0)
                        em = wp.tile([P, HC], f32, name="em", tag="em",
                                     bufs=2)
                        nc.vector.tensor_scalar(
                            out=em[:],
                            in0=ee[:],
                            scalar1=-1.0,
                            scalar2=0.0,
                            op0=OP.add,
                            op1=OP.min,
                        )
                        of = wp.tile([P, HC], f32, name="of", tag="of",
                                     bufs=2)
                        nc.vector.scalar_tensor_tensor(
                            out=of[:],
                            in0=zn.rearrange("p h c -> p (h c)"),
                            scalar=0.0,
                            in1=em[:],
                            op0=OP.max,
                            op1=OP.add,
                        )
                        nc.sync.dma_start(
                            out_d[jt * P:(jt + 1) * P, :], of[:]
                        )

                # ---------------- stage 2: remaining head pairs ----------
                for hp in range(4):
                    h0, h1 = 2 * hp, 2 * hp + 1
                    if hp == 0:
                        nts = nts0
                    else:
                        nts = {
                            h: ntp.tile([65, NJ], f32, name=f"nt{h}", tag="nt")
                            for h in (h0, h1)
                        }
                    for ibp in range(NIB // 2):
                        if hp == 0:
                            break
                        u4 = wp.tile([P, 2, 2, NJ], bf16, name="u4",
                                     tag="u2", bufs=3)
                        for ki in range(2):
                            ib = 2 * ibp + ki
                            for k, h in enumerate((h0, h1)):
                                if (h, ib) in R_TILES:
                                    # ACT route: u = exp(max(0.8a, -0.8b))
                                    r = wp.tile(
                                        [P, NJ], f32, name="r", tag="r", bufs=2
                                    )
                                    nc.scalar.activation(
                                        r[:], bneg[h - 2][:], AF.Relu,
                                        bias=a08n_sb[:, ib, h:h + 1],
                                    )
                                    nc.scalar.activation(
                                        u4[:, ki, k, :], r[:], AF.Exp,
                                        bias=a08p_sb[:, ib, h:h + 1],
                                    )
                                else:
                                    nc.vector.tensor_scalar(
                                        out=u4[:, ki, k, :],
                                        in0=e1rb[h][:],
                                        scalar1=th_sb[:, ib, h:h + 1],
                                        scalar2=None,
                                        op0=OP.max,
                                        op1=OP.bypass,
                                    )
                        pe4 = wp.tile([P, 2, 2, NJ], bf16, name="pe4",
                                      tag="pe", bufs=4)
                        nc.vector.tensor_mul(
                            pe4[:], u4[:],
                            mask_all[:, 2 * ibp:2 * ibp + 2, :].unsqueeze(2)
                            .to_broadcast((P, 2, 2, NJ)),
                        )
                        for ki in range(2):
                            for k, h in enumerate((h0, h1)):
                                for jc in range(2):
                                    nc.tensor.matmul(
                                        nts[h][:, jc * 512:(jc + 1) * 512],
                                        xs2[ibp][:, ki, h, 0:65],
                                        pe4[:, ki, k, jc * 512:(jc + 1) * 512],
                                        start=(ibp == 0 and ki == 0),
                                        stop=(ibp == NIB // 2 - 1 and ki == 1),
                                    )
                    # post: transpose to [j, c]; collect denominators.
                    # jc-major so the final head pair releases the first
                    # j-half early and stage 3 can overlap the second half.
                    nt_sbs = {}
                    for h in (h0, h1):
                        nt_sb = wp.tile([65, NJ], f32, name="nt_sb",
                                        tag="ntsb", bufs=2)
                        nc.scalar.copy(nt_sb[:], nts[h][:])
                        nt_sbs[h] = nt_sb
                    for jc in range(2):
                        for h in (h0, h1):
                            ptq4 = trp.tile([P, 4, 65], f32, name="ptq4",
                                            tag="tr")
                            for jq in range(4):
                                nc.tensor.transpose(
                                    ptq4[:, jq, :],
                                    nt_sbs[h][:, (jc * 4 + jq) * P:
                                              (jc * 4 + jq + 1) * P],
                                    ident[0:65, 0:65],
                                )
                            with nc.allow_low_precision(
                                reason="softmax denominators tolerate bf16"
                            ):
                                nc.vector.reciprocal(
                                    rec_all[:, h, jc * 4:(jc + 1) * 4],
                                    ptq4[:, :, 64],
                                )
                            nc.scalar.copy(
                                zt_all[:, jc * 4:(jc + 1) * 4,
                                       h * C:(h + 1) * C],
                                ptq4[:, :, 0:64],
                            )
                        if hp == 3:
                            _emit_stage3(range(jc * 4, (jc + 1) * 4))

                # ---------------- stage 3: normalize + ELU + store ----------
                pass  # stage 3 emitted inside the hp==3 post loop

    _strip_redundant_pe_waits(nc)
    _split_excess_waits(nc)
    return nc


# empirical per-engine sync-wait budgets in the walrus CoreV3 lowering
_WAIT_BUDGET = {
    "EngineType.PE": 1,
    "EngineType.Activation": 1,
    "EngineType.DVE": 1,
    "EngineType.Pool": 1,
    "EngineType.SP": 1,
}


def _split_excess_waits(nc):
    """Instructions whose on_wait exceeds the engine's wait budget get the
    excess waits moved onto NoOp instructions inserted just before them in
    the same (in-order) engine queue."""
    import concourse.mybir as mybir

    fn = nc.m.functions[0]
    n = 0
    for blk in fn.blocks:
        insts = blk.instructions
        k = 0
        while k < len(insts):
            i = insts[k]
            eng = str(getattr(i, "engine", ""))
            si = getattr(i, "sync_info", None)
            budget = _WAIT_BUDGET.get(eng)
            if type(i).__name__ == "InstTensorScalarPtr":
                # S2S2D2_STT lowering (CoreV2 path) allows only one wait
                budget = 1
            if si is None or budget is None or len(si.on_wait) <= budget:
                k += 1
                continue
            ws = list(si.on_wait)
            excess, keep = ws[: len(ws) - budget], ws[len(ws) - budget:]
            for w in excess:
                nop = mybir.InstNoOp(name=f"I-wsplit{n}", ins=[], outs=[])
                n += 1
                nop.engine = i.engine
                nop.sync_info = type(si)(on_wait=[w], on_update=[])
                insts.insert(k, nop)
                k += 1
            si.on_wait = keep
            i.sync_info = si
            k += 1


def _strip_redundant_pe_waits(nc):
    """walrus allows only ONE sync wait per PE instruction. Tile emits
    [bank-reader-sem, PE-self-sem] pairs on PSUM slot reuse even though the
    reader wait transitively implies the PE WAW wait (the reader itself
    waited for the PE chain). Compute, per instruction in scheduled order,
    the PE tick each semaphore value transitively certifies, and drop PE
    self-waits that are covered by a co-occurring wait."""
    fn = nc.m.functions[0]
    flat = [i for blk in fn.blocks for i in blk.instructions]

    def _merge(dst, src):
        for k, v in src.items():
            if dst.get(k, 0) < v:
                dst[k] = v

    # engine -> its own completion semaphore (each engine executes its
    # instruction stream strictly in order, so waits on the engine's own
    # sem are always satisfied at dispatch and can be dropped)
    self_sem = {}
    for i in flat:
        si = getattr(i, "sync_info", None)
        eng = str(getattr(i, "engine", ""))
        if si is None or type(i).__name__ in ("InstNop", "InstDrain"):
            continue
        if eng not in self_sem and si.on_update:
            nm = si.on_update[0].ant_name
            if not nm.startswith(("DMAHW", "DMASW", "barrier")):
                self_sem[eng] = nm

    obs = {}        # engine -> observed vector clock {sem: tick}
    events = {}     # (sem, value) -> vector clock certified when sem hit value
    sem_val = {}
    for i in flat:
        eng = str(getattr(i, "engine", ""))
        si = getattr(i, "sync_info", None)
        if si is None:
            continue
        o = obs.setdefault(eng, {})
        for w in si.on_wait:
            if w.wait_value is None:
                continue
            if o.get(w.ant_name, 0) < w.wait_value:
                o[w.ant_name] = w.wait_value
            _merge(o, events.get((w.ant_name, w.wait_value), {}))
        if any(w.ant_name == self_sem.get(eng) for w in si.on_wait):
            si.on_wait = [
                w for w in si.on_wait if w.ant_name != self_sem.get(eng)
            ]
            i.sync_info = si
        if len(si.on_wait) > 1:
            ws = [w for w in si.on_wait]
            certs = []
            for w in ws:
                c = dict(events.get((w.ant_name, w.wait_value), {})) \
                    if w.wait_value is not None else {}
                if w.wait_value is not None:
                    c[w.ant_name] = max(c.get(w.ant_name, 0), w.wait_value)
                certs.append(c)
            # greedily keep waits not covered by the union of kept certs
            order = sorted(range(len(ws)), key=lambda j: -len(certs[j]))
            kept, covered = [], {}
            for j in order:
                w = ws[j]
                if (
                    w.wait_value is not None
                    and covered.get(w.ant_name, 0) >= w.wait_value
                ):
                    continue
                kept.append(j)
                _merge(covered, certs[j])
            if len(kept) < len(ws):
                si.on_wait = [ws[j] for j in sorted(kept)]
                i.sync_info = si
        for u in si.on_update:
            if u.update_value is None:
                continue
            v1 = sem_val.get(u.ant_name, 0) + u.update_value
            sem_val[u.ant_name] = v1
            cert = dict(o)
            cert[u.ant_name] = max(cert.get(u.ant_name, 0), v1)
            for vv in range(v1 - u.update_value + 1, v1 + 1):
                events[(u.ant_name, vv)] = cert
            if o.get(u.ant_name, 0) < v1:
                o[u.ant_name] = v1


def _get_program():
    global _PROG
    if _PROG is None:
        _PROG = _build_program()
    return _PROG


def _make_in_maps(features_batch, adj_mats_batch, W, att_src, att_dst, bias):
    import ml_dtypes

    bf = ml_dtypes.bfloat16
    f = np.asarray(features_batch, dtype=np.float32)
    adj = np.asarray(adj_mats_batch, dtype=np.int32)
    Wn = np.ascontiguousarray(np.asarray(W, dtype=np.float32))
    asv = np.asarray(att_src, dtype=np.float32).reshape(H, C)
    adv = np.asarray(att_dst, dtype=np.float32).reshape(H, C)
    bv = np.ascontiguousarray(np.asarray(bias, dtype=np.float32).reshape(1, HC))

    # wa[d, h] = sum_c W[d, h*C+c] * att[h, c]
    W3 = Wn.reshape(D, H, C)
    wa_s = np.einsum("dhc,hc->dh", W3, asv)  # [D, H]
    wa_d = np.einsum("dhc,hc->dh", W3, adv)

    def col_layout(v):
        # [N, H] -> [P, NIB*H]: row p, col (ib*H + h) <- node ib*P+p
        return np.ascontiguousarray(
            v.reshape(NIB, P, H).transpose(1, 0, 2).reshape(P, NIB * H)
        )

    ident = np.eye(P, dtype=np.float32)
    in_maps = []
    for c in range(NCORES):
        b, half = divmod(c, 2)
        j0 = half * NJ
        fb = f[b]                                  # [N, D]
        a_src = fb @ wa_s                          # [N, H]
        a_dst = fb @ wa_d
        bd = -0.8 * a_dst[j0:j0 + NJ, :]           # [NJ, H]
        m = (adj[b][:, j0:j0 + NJ] != 0)
        jdx = np.arange(NJ)
        m[j0 + jdx, jdx] = True                    # self-loops
        in_maps.append(
            {
                "fT": np.ascontiguousarray(fb.T.astype(bf)),
                "W": Wn.astype(bf),
                "Wb": bv.astype(bf),
                "mask": np.ascontiguousarray(m.astype(bf)),
                "e1r": np.ascontiguousarray(np.exp(bd).T.astype(bf)),
                "b08n": np.ascontiguousarray(bd.T.astype(bf)),
                "th": col_layout(np.exp(0.8 * a_src)),
                "e02": col_layout(np.exp(0.2 * a_src)),
                "a08p": col_layout(0.8 * a_src),
                "a08n": col_layout(-0.8 * a_src),
                "ident": ident,
            }
        )
    return in_maps


_RUNNER = None  # cached (jitted_fn, in_names, out_names, n_params, zero_outs)


def _get_runner():
    """Build a jitted shard_map runner for the bass program (mirrors
    concourse.bass2jax.run_bass_via_pjrt but without output donation, so
    device-resident inputs can be reused across timed iterations)."""
    global _RUNNER
    if _RUNNER is not None:
        return _RUNNER
    import jax
    import concourse.mybir as mybir
    from concourse import bass2jax
    from jax.sharding import Mesh, PartitionSpec
    from jax.experimental.shard_map import shard_map

    bass2jax.install_neuronx_cc_hook()
    nc = _get_program()

    partition_name = (
        nc.partition_id_tensor.name if nc.partition_id_tensor else None
    )
    in_names, out_names, out_avals, zero_outs = [], [], [], []
    for alloc in nc.m.functions[0].allocations:
        if not isinstance(alloc, mybir.MemoryLocationSet):
            continue
        name = alloc.memorylocations[0].name
        if alloc.kind == "ExternalInput":
            if name != partition_name:
                in_names.append(name)
        elif alloc.kind == "ExternalOutput":
            shape = tuple(alloc.tensor_shape)
            dtype = mybir.dt.np(alloc.dtype)
            out_names.append(name)
            out_avals.append(jax.core.ShapedArray(shape, dtype))
            zero_outs.append(np.zeros(shape, dtype))
    n_params = len(in_names)
    all_names = in_names + out_names
    if partition_name is not None:
        all_names = all_names + [partition_name]

    def _body(*args):
        operands = list(args)
        if partition_name is not None:
            operands.append(bass2jax.partition_id_tensor())
        outs = bass2jax._bass_exec_p.bind(
            *operands,
            out_avals=tuple(out_avals),
            in_names=tuple(all_names),
            out_names=tuple(out_names),
            lowering_input_output_aliases=(),
            sim_require_finite=True,
            sim_require_nnan=True,
            nc=nc,
        )
        return tuple(outs)

    devices = jax.devices()[:NCORES]
    mesh = Mesh(np.asarray(devices), ("core",))
    n_args = n_params + len(out_names)
    jitted = jax.jit(
        shard_map(
            _body,
            mesh=mesh,
            in_specs=(PartitionSpec("core"),) * n_args,
            out_specs=(PartitionSpec("core"),) * len(out_names),
            check_rep=False,
        ),
        keep_unused=True,
    )
    _RUNNER = (jitted, in_names, out_names, n_params, zero_outs)
    return _RUNNER


def _sharded_device_put(concat_in):
    import jax
    from jax.sharding import Mesh, PartitionSpec, NamedSharding

    devices = jax.devices()[:NCORES]
    mesh = Mesh(np.asarray(devices), ("core",))
    sh = NamedSharding(mesh, PartitionSpec("core"))
    return jax.device_put(concat_in, sh)


def make_device_runner(inputs_dict):
    """Build (run_once, out_check): one warm 8-core inference on
    device-resident pre-sharded inputs, and an output assembler."""
    import jax

    in_maps = _make_in_maps(**inputs_dict)
    jitted, in_names, out_names, n_params, zero_outs = _get_runner()
    concat_in = [
        np.concatenate([m[name] for m in in_maps], axis=0) for name in in_names
    ] + [
        np.concatenate([z] * NCORES, axis=0) for z in zero_outs
    ]
    dev_in = _sharded_device_put(concat_in)

    def run_once():
        outs = jitted(*dev_in)
        jax.block_until_ready(outs)
        return outs

    def out_check(outs):
        np_outs = [np.asarray(o) for o in outs]
        results = [
            {
                name: np_outs[i][c * NJ:(c + 1) * NJ]
                for i, name in enumerate(out_names)
            }
            for c in range(NCORES)
        ]
        return _assemble(results)

    return run_once, out_check


def _run(in_maps, time_iters=0):
    """Execute on 8 cores. Returns (results_list, min_wall_ns or None)."""
    import jax

    jitted, in_names, out_names, n_params, zero_outs = _get_runner()
    concat_in = [
        np.concatenate([m[name] for m in in_maps], axis=0) for name in in_names
    ] + [
        np.concatenate([z] * NCORES, axis=0) for z in zero_outs
    ]
    dev_in = _sharded_device_put(concat_in)
    outs = jitted(*dev_in)
    jax.block_until_ready(outs)

    best_ns = None
    if time_iters > 0:
        import time as _time

        for _ in range(time_iters):
            t0 = _time.perf_counter()
            outs2 = jitted(*dev_in)
            jax.block_until_ready(outs2)
            dt = (_time.perf_counter() - t0) * 1e9
            best_ns = dt if best_ns is None else min(best_ns, dt)
        outs = outs2

    results = []
    np_outs = [np.asarray(o) for o in outs]
    per_core = NJ  # axis-0 length of each core's "out"
    for c in range(NCORES):
        results.append(
            {
                name: np_outs[i][c * per_core:(c + 1) * per_core]
                for i, name in enumerate(out_names)
            }
        )
    return results, best_ns


def _assemble(results):
    out = np.empty((B, N, HC), dtype=np.float32)
    for c in range(NCORES):
        b, half = divmod(c, 2)
        j0 = half * NJ
        out[b, j0:j0 + NJ, :] = results[c]["out"]
    return out


def kernel(features_batch, adj_mats_batch, W, att_src, att_dst, bias):
    in_maps = _make_in_maps(
        features_batch, adj_mats_batch, W, att_src, att_dst, bias
    )
    results, _ = _run(in_maps)
    return _assemble(results)


def run_profiled(features_batch, adj_mats_batch, W, att_src, att_dst, bias,
                 time_iters=10):
    """Like kernel() but also times warm executions; returns (out, min_ns)."""
    in_maps = _make_in_maps(
        features_batch, adj_mats_batch, W, att_src, att_dst, bias
    )
    results, best_ns = _run(in_maps, time_iters=time_iters)
    return _assemble(results), best_ns


# revision 23
# speedup vs baseline: 1.1365x; 1.1365x over previous
"""Trainium2 Bass kernel: batched dense GAT (PyG GATConv, eval, concat heads).

Contract: kernel(**inputs) takes FULL inputs (numpy/jax arrays) and returns the
FULL output [B, N, H*C] float32. Internally shards across 8 NeuronCores:
core c handles graph b = c//2, target-node half j0 = (c%2)*1024.

Math (per graph):
  x = f @ W                       [N, H, C]
  a_src[i,h] = <x[i,h,:], att_src[h,:]>,  a_dst likewise
  logit[i,j,h] = leaky_relu(a_src[i,h] + a_dst[j,h], 0.2), -inf off-mask
  alpha = softmax over i (sources);  out[j] = sum_i alpha * x[i]  (+bias, ELU)

Algorithm: the softmax over i cancels any per-target factor, so
P'' = P / exp(b_j) is used, with
  exp(lrelu(a+b) - b) = max(e^a, e^{0.2a} e^{-0.8b})
                      = e^{0.2a} * max(e^{0.8a}, e^{-0.8b}).
The per-SOURCE factor e^{0.2a} is folded into the x-matrix
(xs = (x + bias) * e^{0.2a}; the ones-column becomes e^{0.2a} so the
denominator falls out of the same matmul, and folding the output bias into
x works because out = num/den + bias = (num + bias*den)/den).

Per (h, ib) attention tile, two route choices feeding a shared u-buffer:
  TS route (DVE): u = (e1rb[h] max th[i])             tensor_scalar, 2x mode
  R route (ACT):  r = relu(-0.8b - 0.8a) ; u = exp(r + 0.8a)   two ACTIVATEs
then ONE mask multiply per head-PAIR (amortizes DVE op overhead):
  pe2 = u2 * mask                tensor_tensor [P, 2*NJ], 2x mode
The aggregation runs one wide [65, 1024] matmul per (h, ib) accumulating
[num | den] over i-blocks; PE transposes (4 per bank) land in a single PSUM
bank so one strided ACT copy moves 4 j-quads at once; normalization is a
stride-0-broadcast reciprocal multiply at stage 3.

Host precomputes a_src/a_dst (= f @ (W att)) and ships exp families (tiny),
bf16 fT/W, and the bf16 0/1 self-loop mask (no device casts, half the DMA).
"""

import sys

for _p in ("/opt/trn_rl_repo",):
    if _p not in sys.path:
        sys.path.insert(0, _p)

import numpy as np

# Problem dims (fixed by the task)
B, N, D = 4, 2048, 512
H, C = 8, 64
HC = H * C
NCORES = 8
NJ = 1024        # target columns per core
P = 128
NIB = N // P     # 16 source blocks
NJB = NJ // P    # 8 target blocks
KD = D // P      # 4 contraction blocks
SLOPE = 0.2
FB = 66          # per-head xs stride: 64 x-cols + e02 col + pad (4B align)

# (h, ib) tiles routed through the ACT-heavy R route (Relu+Exp with the
# 0.8a terms as ACT biases), spread evenly over heads 2..7 so the all-DVE
# warm-up pair (heads 0-1) can be emitted before stage 1.
N_R_TILES = 34
_r_pool = [(h, ib) for h in range(2, H) for ib in range(1, NIB - 1)]
R_TILES = frozenset(
    _r_pool[(k * len(_r_pool)) // N_R_TILES] for k in range(N_R_TILES)
)

_PROG = None  # cached (nc, input_names)


def _build_program():
    import concourse.bass as bass
    import concourse.mybir as mybir
    import concourse.tile as tile

    f32 = mybir.dt.float32
    bf16 = mybir.dt.bfloat16
    AF = mybir.ActivationFunctionType
    OP = mybir.AluOpType

    nc = bass.Bass("TRN2", target_bir_lowering=False, debug=False)

    fT_d = nc.dram_tensor("fT", [D, N], bf16, kind="ExternalInput").ap()
    W_d = nc.dram_tensor("W", [D, HC], bf16, kind="ExternalInput").ap()
    Wb_d = nc.dram_tensor("Wb", [1, HC], bf16, kind="ExternalInput").ap()
    # 0/1 mask with self-loops ORed in, bf16
    mask_d = nc.dram_tensor("mask", [N, NJ], bf16, kind="ExternalInput").ap()
    # e1r[h, j] = exp(-0.8 a_dst[j,h]);  b08n[h, j] = -0.8 a_dst[j,h]
    e1r_d = nc.dram_tensor("e1r", [H, NJ], bf16, kind="ExternalInput").ap()
    b08n_d = nc.dram_tensor("b08n", [H, NJ], bf16, kind="ExternalInput").ap()
    # per-source scalars, [p, ib*H + h] layout:
    # th = exp(0.8 a_src), e02 = exp(0.2 a_src), a08p/a08n = +/-0.8 a_src
    th_d = nc.dram_tensor("th", [P, NIB * H], f32, kind="ExternalInput").ap()
    e02_d = nc.dram_tensor("e02", [P, NIB * H], f32, kind="ExternalInput").ap()
    a08p_d = nc.dram_tensor("a08p", [P, NIB * H], f32, kind="ExternalInput").ap()
    a08n_d = nc.dram_tensor("a08n", [P, NIB * H], f32, kind="ExternalInput").ap()
    ident_d = nc.dram_tensor("ident", [P, P], f32, kind="ExternalInput").ap()
    out_d = nc.dram_tensor("out", [NJ, HC], f32, kind="ExternalOutput").ap()

    with tile.TileContext(nc) as tc:
        with (
            tc.tile_pool(name="persist", bufs=1) as pp,
            tc.tile_pool(name="psum_nt", bufs=2, space="PSUM") as ntp,
            tc.tile_pool(name="psum_tr", bufs=2, space="PSUM") as trp,
        ):
            # ---------------- persistent inputs ----------------
            # sync (SP) DMA queue: projection weights first (PE critical
            # path), then the big mask tiles.  gpsimd (Pool) queue: all the
            # small/broadcast tensors the DVE/ACT need early.
            th_sb = pp.tile([P, NIB, H], f32, name="th_sb")
            nc.sync.dma_start(
                th_sb[:], th_d.rearrange("p (a h) -> p a h", a=NIB)
            )
            e1rb = [pp.tile([P, NJ], bf16, name=f"e1rb{h}") for h in range(H)]
            for h in range(2):
                nc.sync.dma_start(
                    e1rb[h][:], e1r_d[h:h + 1, :].to_broadcast((P, NJ))
                )
            e02_sb = pp.tile([P, NIB, H], f32, name="e02_sb")
            nc.gpsimd.dma_start(
                e02_sb[:], e02_d.rearrange("p (a h) -> p a h", a=NIB)
            )
            ident = pp.tile([P, P], f32, name="ident")
            nc.gpsimd.dma_start(ident[:], ident_d[:])
            a08p_sb = pp.tile([P, NIB, H], f32, name="a08p_sb")
            nc.gpsimd.dma_start(
                a08p_sb[:], a08p_d.rearrange("p (a h) -> p a h", a=NIB)
            )
            a08n_sb = pp.tile([P, NIB, H], f32, name="a08n_sb")
            nc.gpsimd.dma_start(
                a08n_sb[:], a08n_d.rearrange("p (a h) -> p a h", a=NIB)
            )
            for h in range(2, H):
                nc.gpsimd.dma_start(
                    e1rb[h][:], e1r_d[h:h + 1, :].to_broadcast((P, NJ))
                )
            bneg = [pp.tile([P, NJ], bf16, name=f"bneg{h}") for h in range(2, H)]
            for h in range(2, H):
                nc.gpsimd.dma_start(
                    bneg[h - 2][:], b08n_d[h:h + 1, :].to_broadcast((P, NJ))
                )
            Wb = pp.tile([1, HC], bf16, name="Wb")
            nc.gpsimd.dma_start(Wb[:], Wb_d[:])

            mask_all = pp.tile([P, NIB, NJ], bf16, name="mask_all")

            junk_ps = ntp.tile([1, 1], f32, name="junk_ps", tag="nt")

            # persistent cross-stage tensors
            xs2 = [pp.tile([P, 2, H, FB], bf16, name=f"xs{i}")
                   for i in range(NIB // 2)]
            zt_all = pp.tile([P, NJB, HC], bf16, name="zt_all")
            rec_all = pp.tile([P, H, NJB], bf16, name="rec_all")
            onesr = pp.tile([1, P], bf16, name="onesr")
            nc.vector.memset(onesr[:], 1.0)

            with (
                tc.tile_pool(name="stage1", bufs=1) as s1p,
                tc.tile_pool(name="work1", bufs=3) as wp1,
                tc.tile_pool(name="work2", bufs=3) as wp,
                tc.tile_pool(name="psum_x", bufs=2, space="PSUM") as xp,
            ):
                # sync queue order tuned for pipeline fill: the first two
                # mask tiles (DVE warm-up), then fT COLUMN chunks q=0 for all
                # kd (+W) so projection of the first i-blocks can start
                # before the whole fT lands, then masks interleaved with the
                # remaining fT column chunks.
                fTt = [s1p.tile([P, N], bf16, name=f"ft_{kd}")
                       for kd in range(KD)]
                Wt = [s1p.tile([P, HC], bf16, name=f"w_{kd}")
                      for kd in range(KD)]
                for ib in range(2):
                    nc.sync.dma_start(
                        mask_all[:, ib, :], mask_d[ib * P:(ib + 1) * P, :]
                    )
                NQ = N // 4
                for kd in range(KD):
                    q0 = nc.sync if kd < 2 else nc.scalar
                    q0.dma_start(
                        fTt[kd][:, 0:NQ], fT_d[kd * P:(kd + 1) * P, 0:NQ]
                    )
                    q0.dma_start(Wt[kd][:], W_d[kd * P:(kd + 1) * P, :])
                for q in range(1, 4):
                    for kd in range(KD):
                        nc.scalar.dma_start(
                            fTt[kd][:, q * NQ:(q + 1) * NQ],
                            fT_d[kd * P:(kd + 1) * P, q * NQ:(q + 1) * NQ],
                        )
                for ib in range(2, NIB):
                    nc.sync.dma_start(
                        mask_all[:, ib, :], mask_d[ib * P:(ib + 1) * P, :]
                    )

                # PE-touch the DMA-loaded ident early (single-wait PE rule:
                # later transposes reusing it then need no DMA wait).
                nc.tensor.matmul(
                    junk_ps[:], ident[0:1, 0:1], ident[0:1, 0:1],
                    start=True, stop=True,
                )

                # -------- stage 2 warm-up: heads 0-1 u/pe quads for the
                # first PREQ i-block PAIRS (all-DVE TS route) emitted FIRST
                # so the DVE queue has runway while PE does the projection.
                # PREQ must stay below the pe pool depth: later pe tiles
                # recycle buffers whose consumers (agg matmuls) are emitted
                # after stage 1, so emitting more would deadlock the DVE
                # queue.
                PREQ = 4
                pe_hp0 = []
                for ibp in range(PREQ):
                    u4 = wp.tile([P, 2, 2, NJ], bf16, name="u4", tag="u2",
                                 bufs=3)
                    for ki in range(2):
                        for k in range(2):
                            nc.vector.tensor_scalar(
                                out=u4[:, ki, k, :],
                                in0=e1rb[k][:],
                                scalar1=th_sb[:, 2 * ibp + ki, k:k + 1],
                                scalar2=None,
                                op0=OP.max,
                                op1=OP.bypass,
                            )
                    pe4 = wp.tile([P, 2, 2, NJ], bf16, name="pe4", tag="pe",
                                  bufs=4)
                    nc.vector.tensor_mul(
                        pe4[:], u4[:],
                        mask_all[:, 2 * ibp:2 * ibp + 2, :].unsqueeze(2)
                        .to_broadcast((P, 2, 2, NJ)),
                    )
                    pe_hp0.append(pe4)

                # ---------------- fused stage 1 + head-pair 0 ----------
                # per i-block: projection matmuls -> xs, then immediately the
                # hp0 aggregation matmuls, so PE never serializes a long
                # projection-only phase while DVE runs ahead.
                nts0 = {
                    h: ntp.tile([65, NJ], f32, name=f"nt{h}", tag="nt")
                    for h in (0, 1)
                }
                for ibp in range(NIB // 2):
                    tmp = wp1.tile([P, 2, H, FB], bf16, name="tmp", tag="tmp")
                    for ki in range(2):
                        ib = 2 * ibp + ki
                        px = xp.tile([P, HC], f32, name="px", tag="px")
                        for kd in range(KD):
                            nc.tensor.matmul(
                                px[:],
                                fTt[kd][:, ib * P:(ib + 1) * P],
                                Wt[kd][:],
                                start=(kd == 0),
                                stop=False,
                            )
                        # bias via rank-1 ones matmul (output bias -> x)
                        nc.tensor.matmul(
                            px[:], onesr[:], Wb[:], start=False, stop=True,
                        )
                        nc.scalar.copy(
                            tmp[:, ki, :, 0:64],
                            px.rearrange("p (h c) -> p h c", h=H),
                        )
                    nc.vector.memset(tmp[:, :, :, 64:FB], 1.0)
                    e02b = e02_sb[:, 2 * ibp:2 * ibp + 2, :].unsqueeze(
                        3).to_broadcast((P, 2, H, FB))
                    xsp = xs2[ibp]
                    nc.vector.tensor_mul(xsp[:], tmp[:], e02b)
                    if ibp < PREQ:
                        pe4 = pe_hp0[ibp]
                    else:
                        u4 = wp.tile([P, 2, 2, NJ], bf16, name="u4",
                                     tag="u2", bufs=3)
                        for ki in range(2):
                            for k in range(2):
                                nc.vector.tensor_scalar(
                                    out=u4[:, ki, k, :],
                                    in0=e1rb[k][:],
                                    scalar1=th_sb[:, 2 * ibp + ki, k:k + 1],
                                    scalar2=None,
                                    op0=OP.max,
                                    op1=OP.bypass,
                                )
                        pe4 = wp.tile([P, 2, 2, NJ], bf16, name="pe4",
                                      tag="pe", bufs=4)
                        nc.vector.tensor_mul(
                            pe4[:], u4[:],
                            mask_all[:, 2 * ibp:2 * ibp + 2, :].unsqueeze(2)
                            .to_broadcast((P, 2, 2, NJ)),
                        )
                    for ki in range(2):
                        for k, h in enumerate((0, 1)):
                            for jc in range(2):
                                nc.tensor.matmul(
                                    nts0[h][:, jc * 512:(jc + 1) * 512],
                                    xsp[:, ki, h, 0:65],
                                    pe4[:, ki, k, jc * 512:(jc + 1) * 512],
                                    start=(ibp == 0 and ki == 0),
                                    stop=(ibp == NIB // 2 - 1 and ki == 1),
                                )

                # ---------------- stage 2: remaining head pairs ----------
                for hp in range(4):
                    h0, h1 = 2 * hp, 2 * hp + 1
                    if hp == 0:
                        nts = nts0
                    else:
                        nts = {
                            h: ntp.tile([65, NJ], f32, name=f"nt{h}", tag="nt")
                            for h in (h0, h1)
                        }
                    for ibp in range(NIB // 2):
                        if hp == 0:
                            break
                        u4 = wp.tile([P, 2, 2, NJ], bf16, name="u4",
                                     tag="u2", bufs=3)
                        for ki in range(2):
                            ib = 2 * ibp + ki
                            for k, h in enumerate((h0, h1)):
                                if (h, ib) in R_TILES:
                                    # ACT route: u = exp(max(0.8a, -0.8b))
                                    r = wp.tile(
                                        [P, NJ], f32, name="r", tag="r", bufs=2
                                    )
                                    nc.scalar.activation(
                                        r[:], bneg[h - 2][:], AF.Relu,
                                        bias=a08n_sb[:, ib, h:h + 1],
                                    )
                                    nc.scalar.activation(
                                        u4[:, ki, k, :], r[:], AF.Exp,
                                        bias=a08p_sb[:, ib, h:h + 1],
                                    )
                                else:
                                    nc.vector.tensor_scalar(
                                        out=u4[:, ki, k, :],
                                        in0=e1rb[h][:],
                                        scalar1=th_sb[:, ib, h:h + 1],
                                        scalar2=None,
                                        op0=OP.max,
                                        op1=OP.bypass,
                                    )
                        pe4 = wp.tile([P, 2, 2, NJ], bf16, name="pe4",
                                      tag="pe", bufs=4)
                        nc.vector.tensor_mul(
                            pe4[:], u4[:],
                            mask_all[:, 2 * ibp:2 * ibp + 2, :].unsqueeze(2)
                            .to_broadcast((P, 2, 2, NJ)),
                        )
                        for ki in range(2):
                            for k, h in enumerate((h0, h1)):
                                for jc in range(2):
                                    nc.tensor.matmul(
                                        nts[h][:, jc * 512:(jc + 1) * 512],
                                        xs2[ibp][:, ki, h, 0:65],
                                        pe4[:, ki, k, jc * 512:(jc + 1) * 512],
                                        start=(ibp == 0 and ki == 0),
                                        stop=(ibp == NIB // 2 - 1 and ki == 1),
                                    )
                    # post: transpose to [j, c]; collect denominators
                    for h in (h0, h1):
                        nt_sb = wp.tile([65, NJ], f32, name="nt_sb", tag="ntsb", bufs=2)
                        nc.scalar.copy(nt_sb[:], nts[h][:])
                        for jc in range(2):
                            ptq4 = trp.tile([P, 4, 65], f32, name="ptq4",
                                            tag="tr")
                            for jq in range(4):
                                nc.tensor.transpose(
                                    ptq4[:, jq, :],
                                    nt_sb[:, (jc * 4 + jq) * P:
                                          (jc * 4 + jq + 1) * P],
                                    ident[0:65, 0:65],
                                )
                            with nc.allow_low_precision(
                                reason="softmax denominators tolerate bf16"
                            ):
                                nc.vector.reciprocal(
                                    rec_all[:, h, jc * 4:(jc + 1) * 4],
                                    ptq4[:, :, 64],
                                )
                            nc.scalar.copy(
                                zt_all[:, jc * 4:(jc + 1) * 4,
                                       h * C:(h + 1) * C],
                                ptq4[:, :, 0:64],
                            )

                # ---------------- stage 3: normalize + ELU + store ----------
                for jt in range(NJB):
                    recb = rec_all[:, :, jt].unsqueeze(2).to_broadcast(
                        (P, H, C)
                    )
                    zn = wp.tile([P, H, C], bf16, name="zn", tag="zn", bufs=2)
                    nc.vector.tensor_mul(
                        zn[:], zt_all[:, jt, :].rearrange(
                            "p (h c) -> p h c", h=H), recb,
                    )
                    ee = wp.tile([P, HC], f32, name="ee", tag="ee", bufs=2)
                    nc.scalar.activation(
                        ee[:], zn.rearrange("p h c -> p (h c)"), AF.Exp
                    )
                    # elu(z) = relu(z) + min(exp(z) - 1, 0)
                    em = wp.tile([P, HC], f32, name="em", tag="em", bufs=2)
                    nc.vector.tensor_scalar(
                        out=em[:],
                        in0=ee[:],
                        scalar1=-1.0,
                        scalar2=0.0,
                        op0=OP.add,
                        op1=OP.min,
                    )
                    of = wp.tile([P, HC], f32, name="of", tag="of", bufs=2)
                    nc.vector.scalar_tensor_tensor(
                        out=of[:],
                        in0=zn.rearrange("p h c -> p (h c)"),
                        scalar=0.0,
                        in1=em[:],
                        op0=OP.max,
                        op1=OP.add,
                    )
                    nc.sync.dma_start(out_d[jt * P:(jt + 1) * P, :], of[:])

    _strip_redundant_pe_waits(nc)
    _split_excess_waits(nc)
    return nc


# empirical per-engine sync-wait budgets in the walrus CoreV3 lowering
_WAIT_BUDGET = {
    "EngineType.PE": 1,
    "EngineType.Activation": 1,
    "EngineType.DVE": 1,
    "EngineType.Pool": 1,
    "EngineType.SP": 1,
}


def _split_excess_waits(nc):
    """Instructions whose on_wait exceeds the engine's wait budget get the
    excess waits moved onto NoOp instructions inserted just before them in
    the same (in-order) engine queue."""
    import concourse.mybir as mybir

    fn = nc.m.functions[0]
    n = 0
    for blk in fn.blocks:
        insts = blk.instructions
        k = 0
        while k < len(insts):
            i = insts[k]
            eng = str(getattr(i, "engine", ""))
            si = getattr(i, "sync_info", None)
            budget = _WAIT_BUDGET.get(eng)
            if type(i).__name__ == "InstTensorScalarPtr":
                # S2S2D2_STT lowering (CoreV2 path) allows only one wait
                budget = 1
            if si is None or budget is None or len(si.on_wait) <= budget:
                k += 1
                continue
            ws = list(si.on_wait)
            excess, keep = ws[: len(ws) - budget], ws[len(ws) - budget:]
            for w in excess:
                nop = mybir.InstNoOp(name=f"I-wsplit{n}", ins=[], outs=[])
                n += 1
                nop.engine = i.engine
                nop.sync_info = type(si)(on_wait=[w], on_update=[])
                insts.insert(k, nop)
                k += 1
            si.on_wait = keep
            i.sync_info = si
            k += 1


def _strip_redundant_pe_waits(nc):
    """walrus allows only ONE sync wait per PE instruction. Tile emits
    [bank-reader-sem, PE-self-sem] pairs on PSUM slot reuse even though the
    reader wait transitively implies the PE WAW wait (the reader itself
    waited for the PE chain). Compute, per instruction in scheduled order,
    the PE tick each semaphore value transitively certifies, and drop PE
    self-waits that are covered by a co-occurring wait."""
    fn = nc.m.functions[0]
    flat = [i for blk in fn.blocks for i in blk.instructions]

    def _merge(dst, src):
        for k, v in src.items():
            if dst.get(k, 0) < v:
                dst[k] = v

    # engine -> its own completion semaphore (each engine executes its
    # instruction stream strictly in order, so waits on the engine's own
    # sem are always satisfied at dispatch and can be dropped)
    self_sem = {}
    for i in flat:
        si = getattr(i, "sync_info", None)
        eng = str(getattr(i, "engine", ""))
        if si is None or type(i).__name__ in ("InstNop", "InstDrain"):
            continue
        if eng not in self_sem and si.on_update:
            nm = si.on_update[0].ant_name
            if not nm.startswith(("DMAHW", "DMASW", "barrier")):
                self_sem[eng] = nm

    obs = {}        # engine -> observed vector clock {sem: tick}
    events = {}     # (sem, value) -> vector clock certified when sem hit value
    sem_val = {}
    for i in flat:
        eng = str(getattr(i, "engine", ""))
        si = getattr(i, "sync_info", None)
        if si is None:
            continue
        o = obs.setdefault(eng, {})
        for w in si.on_wait:
            if w.wait_value is None:
                continue
            if o.get(w.ant_name, 0) < w.wait_value:
                o[w.ant_name] = w.wait_value
            _merge(o, events.get((w.ant_name, w.wait_value), {}))
        if any(w.ant_name == self_sem.get(eng) for w in si.on_wait):
            si.on_wait = [
                w for w in si.on_wait if w.ant_name != self_sem.get(eng)
            ]
            i.sync_info = si
        if len(si.on_wait) > 1:
            ws = [w for w in si.on_wait]
            certs = []
            for w in ws:
                c = dict(events.get((w.ant_name, w.wait_value), {})) \
                    if w.wait_value is not None else {}
                if w.wait_value is not None:
                    c[w.ant_name] = max(c.get(w.ant_name, 0), w.wait_value)
                certs.append(c)
            # greedily keep waits not covered by the union of kept certs
            order = sorted(range(len(ws)), key=lambda j: -len(certs[j]))
            kept, covered = [], {}
            for j in order:
                w = ws[j]
                if (
                    w.wait_value is not None
                    and covered.get(w.ant_name, 0) >= w.wait_value
                ):
                    continue
                kept.append(j)
                _merge(covered, certs[j])
            if len(kept) < len(ws):
                si.on_wait = [ws[j] for j in sorted(kept)]
                i.sync_info = si
        for u in si.on_update:
            if u.update_value is None:
                continue
            v1 = sem_val.get(u.ant_name, 0) + u.update_value
            sem_val[u.ant_name] = v1
            cert = dict(o)
            cert[u.ant_name] = max(cert.get(u.ant_name, 0), v1)
            for vv in range(v1 - u.update_value + 1, v1 + 1):
                events[(u.ant_name, vv)] = cert
            if o.get(u.ant_name, 0) < v1:
                o[u.ant_name] = v1


def _get_program():
    global _PROG
    if _PROG is None:
        _PROG = _build_program()
    return _PROG


def _make_in_maps(features_batch, adj_mats_batch, W, att_src, att_dst, bias):
    import ml_dtypes

    bf = ml_dtypes.bfloat16
    f = np.asarray(features_batch, dtype=np.float32)
    adj = np.asarray(adj_mats_batch, dtype=np.int32)
    Wn = np.ascontiguousarray(np.asarray(W, dtype=np.float32))
    asv = np.asarray(att_src, dtype=np.float32).reshape(H, C)
    adv = np.asarray(att_dst, dtype=np.float32).reshape(H, C)
    bv = np.ascontiguousarray(np.asarray(bias, dtype=np.float32).reshape(1, HC))

    # wa[d, h] = sum_c W[d, h*C+c] * att[h, c]
    W3 = Wn.reshape(D, H, C)
    wa_s = np.einsum("dhc,hc->dh", W3, asv)  # [D, H]
    wa_d = np.einsum("dhc,hc->dh", W3, adv)

    def col_layout(v):
        # [N, H] -> [P, NIB*H]: row p, col (ib*H + h) <- node ib*P+p
        return np.ascontiguousarray(
            v.reshape(NIB, P, H).transpose(1, 0, 2).reshape(P, NIB * H)
        )

    ident = np.eye(P, dtype=np.float32)
    in_maps = []
    for c in range(NCORES):
        b, half = divmod(c, 2)
        j0 = half * NJ
        fb = f[b]                                  # [N, D]
        a_src = fb @ wa_s                          # [N, H]
        a_dst = fb @ wa_d
        bd = -0.8 * a_dst[j0:j0 + NJ, :]           # [NJ, H]
        m = (adj[b][:, j0:j0 + NJ] != 0)
        jdx = np.arange(NJ)
        m[j0 + jdx, jdx] = True                    # self-loops
        in_maps.append(
            {
                "fT": np.ascontiguousarray(fb.T.astype(bf)),
                "W": Wn.astype(bf),
                "Wb": bv.astype(bf),
                "mask": np.ascontiguousarray(m.astype(bf)),
                "e1r": np.ascontiguousarray(np.exp(bd).T.astype(bf)),
                "b08n": np.ascontiguousarray(bd.T.astype(bf)),
                "th": col_layout(np.exp(0.8 * a_src)),
                "e02": col_layout(np.exp(0.2 * a_src)),
                "a08p": col_layout(0.8 * a_src),
                "a08n": col_layout(-0.8 * a_src),
                "ident": ident,
            }
        )
    return in_maps


_RUNNER = None  # cached (jitted_fn, in_names, out_names, n_params, zero_outs)


def _get_runner():
    """Build a jitted shard_map runner for the bass program (mirrors
    concourse.bass2jax.run_bass_via_pjrt but without output donation, so
    device-resident inputs can be reused across timed iterations)."""
    global _RUNNER
    if _RUNNER is not None:
        return _RUNNER
    import jax
    import concourse.mybir as mybir
    from concourse import bass2jax
    from jax.sharding import Mesh, PartitionSpec
    from jax.experimental.shard_map import shard_map

    bass2jax.install_neuronx_cc_hook()
    nc = _get_program()

    partition_name = (
        nc.partition_id_tensor.name if nc.partition_id_tensor else None
    )
    in_names, out_names, out_avals, zero_outs = [], [], [], []
    for alloc in nc.m.functions[0].allocations:
        if not isinstance(alloc, mybir.MemoryLocationSet):
            continue
        name = alloc.memorylocations[0].name
        if alloc.kind == "ExternalInput":
            if name != partition_name:
                in_names.append(name)
        elif alloc.kind == "ExternalOutput":
            shape = tuple(alloc.tensor_shape)
            dtype = mybir.dt.np(alloc.dtype)
            out_names.append(name)
            out_avals.append(jax.core.ShapedArray(shape, dtype))
            zero_outs.append(np.zeros(shape, dtype))
    n_params = len(in_names)
    all_names = in_names + out_names
    if partition_name is not None:
        all_names = all_names + [partition_name]

    def _body(*args):
        operands = list(args)
        if partition_name is not None:
            operands.append(bass2jax.partition_id_tensor())
        outs = bass2jax._bass_exec_p.bind(
            *operands,
            out_avals=tuple(out_avals),
            in_names=tuple(all_names),
            out_names=tuple(out_names),
            lowering_input_output_aliases=(),
            sim_require_finite=True,
            sim_require_nnan=True,
            nc=nc,
        )
        return tuple(outs)

    devices = jax.devices()[:NCORES]
    mesh = Mesh(np.asarray(devices), ("core",))
    n_args = n_params + len(out_names)
    jitted = jax.jit(
        shard_map(
            _body,
            mesh=mesh,
            in_specs=(PartitionSpec("core"),) * n_args,
            out_specs=(PartitionSpec("core"),) * len(out_names),
            check_rep=False,
        ),
        keep_unused=True,
    )
    _RUNNER = (jitted, in_names, out_names, n_params, zero_outs)
    return _RUNNER


def _sharded_device_put(concat_in):
    import jax
    from jax.sharding import Mesh, PartitionSpec, NamedSharding

    devices = jax.devices()[:NCORES]
    mesh = Mesh(np.asarray(devices), ("core",))
    sh = NamedSharding(mesh, PartitionSpec("core"))
    return jax.device_put(concat_in, sh)


def make_device_runner(inputs_dict):
    """Build (run_once, out_check): one warm 8-core inference on
    device-resident pre-sharded inputs, and an output assembler."""
    import jax

    in_maps = _make_in_maps(**inputs_dict)
    jitted, in_names, out_names, n_params, zero_outs = _get_runner()
    concat_in = [
        np.concatenate([m[name] for m in in_maps], axis=0) for name in in_names
    ] + [
        np.concatenate([z] * NCORES, axis=0) for z in zero_outs
    ]
    dev_in = _sharded_device_put(concat_in)

    def run_once():
        outs = jitted(*dev_in)
        jax.block_until_ready(outs)
        return outs

    def out_check(outs):
        np_outs = [np.asarray(o) for o in outs]
        results = [
            {
                name: np_outs[i][c * NJ:(c + 1) * NJ]
                for i, name in enumerate(out_names)
            }
            for c in range(NCORES)
        ]
        return _assemble(results)

    return run_once, out_check


def _run(in_maps, time_iters=0):
    """Execute on 8 cores. Returns (results_list, min_wall_ns or None)."""
    import jax

    jitted, in_names, out_names, n_params, zero_outs = _get_runner()
    concat_in = [
        np.concatenate([m[name] for m in in_maps], axis=0) for name in in_names
    ] + [
        np.concatenate([z] * NCORES, axis=0) for z in zero_outs
    ]
    dev_in = _sharded_device_put(concat_in)
    outs = jitted(*dev_in)
    jax.block_until_ready(outs)

    best_ns = None
    if time_iters > 0:
        import time as _time

        for _ in range(time_iters):
            t0 = _time.perf_counter()
            outs2 = jitted(*dev_in)
            jax.block_until_ready(outs2)
            dt = (_time.perf_counter() - t0) * 1e9
            best_ns = dt if best_ns is None else min(best_ns, dt)
        outs = outs2

    results = []
    np_outs = [np.asarray(o) for o in outs]
    per_core = NJ  # axis-0 length of each core's "out"
    for c in range(NCORES):
        results.append(
            {
                name: np_outs[i][c * per_core:(c + 1) * per_core]
                for i, name in enumerate(out_names)
            }
        )
    return results, best_ns


def _assemble(results):
    out = np.empty((B, N, HC), dtype=np.float32)
    for c in range(NCORES):
        b, half = divmod(c, 2)
        j0 = half * NJ
        out[b, j0:j0 + NJ, :] = results[c]["out"]
    return out


def kernel(features_batch, adj_mats_batch, W, att_src, att_dst, bias):
    in_maps = _make_in_maps(
        features_batch, adj_mats_batch, W, att_src, att_dst, bias
    )
    results, _ = _run(in_maps)
    return _assemble(results)


def run_profiled(features_batch, adj_mats_batch, W, att_src, att_dst, bias,
                 time_iters=10):
    """Like kernel() but also times warm executions; returns (out, min_ns)."""
    in_maps = _make_in_maps(
        features_batch, adj_mats_batch, W, att_src, att_dst, bias
    )
    results, best_ns = _run(in_maps, time_iters=time_iters)
    return _assemble(results), best_ns
